# revision 17
# baseline (speedup 1.0000x reference)
"""Trainium2 Bass kernel for nn_BeatsRandomTokenizer (vq_codebook).

Math: the reference chain (patch-conv 256->512, LayerNorm, proj 512->256,
l2norm, cosine-vs-codebook argmax) folds into a single linear map:
  - LN variance scaling and l2norm are positive per-row scalars -> argmax
    invariant.
  - LN mean subtraction is linear -> folds into the weight.
  sim = P @ W4,  W4 = Wc_centered @ proj @ l2norm(codebook).T  (256 x 1024)
  out = argmax_k sim

Device kernel (per core, 2 of 16 batches):
  64 patch-tiles of 128 patches, software-pipelined. Per tile: one 3-dim
  DMA gathers the patch-major tile [128, 256] (64B runs; partition
  p = 16*w + h so the partition dim merges to a single AP dim), the
  hi/lo bf16 split runs on ACT (cast) + GpSimd (subtract), PE transposes
  the bf16 halves to [c, patch], and 12 bf16 matmuls accumulate the
  f32-accurate product (Ph@Wh + Pl@Wh + Ph@Wl) into PSUM. DVE
  max/max_index read PSUM directly for the argmax over the 1024 codes.
  Measured on trn2: ~212 us HW exec, exact vs the f32 reference.
"""
import numpy as np
import ml_dtypes

import concourse.bass as bass
import concourse.mybir as mybir
import concourse.tile as tile
from concourse import bacc
from concourse.bass_utils import run_bass_kernel_spmd
from concourse.masks import make_identity

B, T, M = 16, 8192, 128
PATCH = 16
D, Q, KCODES = 512, 256, 1024
NCORES = 8
B_LOC = B // NCORES            # 2 batches per core
HB = T // PATCH                # 512 h-patches per batch
WB = M // PATCH                # 8 w-patches
NPB = HB * WB                  # 4096 patches per batch
C = PATCH * PATCH              # 256 patch elements
TILES_PER_B = NPB // 128       # 32 patch-tiles per batch
NT = B_LOC * TILES_PER_B       # 64 tiles per core

_CACHE = {}


def _build_nc():
    nc = bacc.Bacc()
    fb_d = nc.declare_dram_parameter("fb", [B_LOC, T, M], mybir.dt.float32,
                                     isOutput=False)
    wh_d = nc.declare_dram_parameter("wh", [128, 2, KCODES], mybir.dt.bfloat16,
                                     isOutput=False)
    wl_d = nc.declare_dram_parameter("wl", [128, 2, KCODES], mybir.dt.bfloat16,
                                     isOutput=False)
    out_d = nc.declare_dram_parameter("out", [128, NT], mybir.dt.uint32,
                                      isOutput=True)

    # [b, w, t, wp]: per tile in_ = fb_w[b, :, th*256:(th+1)*256, :]
    # -> 3-dim AP [[16,8],[128,256],[1,16]]; partition p = 16*w + h_local
    fb_w = fb_d.rearrange("b t (w wp) -> b w t wp", wp=PATCH)

    with tile.TileContext(nc) as tc:
        with tc.tile_pool(name="singles", bufs=1) as singles, \
             tc.tile_pool(name="io", bufs=6) as io, \
             tc.tile_pool(name="work", bufs=3) as work, \
             tc.tile_pool(name="simpool", bufs=3) as simpool, \
             tc.tile_pool(name="pspool", bufs=2, space="PSUM") as pspool, \
             tc.tile_pool(name="simps", bufs=3, space="PSUM") as simps:
            ident = singles.tile([128, 128], mybir.dt.bfloat16)
            make_identity(nc, ident)
            zero1 = singles.tile([128, 1], mybir.dt.float32)
            nc.vector.memset(zero1, 0.0)

            whs = singles.tile([128, 2, KCODES], mybir.dt.bfloat16)
            wls = singles.tile([128, 2, KCODES], mybir.dt.bfloat16)
            nc.sync.dma_start(out=whs, in_=wh_d[:, :, :])
            nc.sync.dma_start(out=wls, in_=wl_d[:, :, :])

            out_sb = singles.tile([128, NT], mybir.dt.uint32)

            # keep the PE HAM warm during the initial DMA fill (and absorb
            # the Pool->PE identity wait once)
            for wi in range(48):
                warm_ps = pspool.tile([128, 4, 128], mybir.dt.bfloat16,
                                      name=f"warm{wi}", tag="pt_ps")
                nc.tensor.transpose(warm_ps[:, 0, :], ident[:, :], ident[:, :])

            def stage_a(t):
                b, th = divmod(t, TILES_PER_B)
                ptile = io.tile([128, C], mybir.dt.float32, name="ptile",
                                tag="ptile")
                dma_eng = nc.sync if t % 2 == 0 else nc.scalar
                dma_eng.dma_start(
                    out=ptile,
                    in_=fb_w[b, :, th * 256:(th + 1) * 256, :])

                # hi/lo bf16 split in [patch, c] layout (ACT cast + Pool sub)
                ph_pc = work.tile([128, C], mybir.dt.bfloat16, name="ph_pc",
                                  tag="ph_pc")
                pl_pc = work.tile([128, C], mybir.dt.bfloat16, name="pl_pc",
                                  tag="pl_pc")
                nc.scalar.activation(ph_pc[:, :], ptile[:, :],
                                     mybir.ActivationFunctionType.Copy)
                nc.gpsimd.tensor_tensor(out=pl_pc[:, :], in0=ptile[:, :],
                                        in1=ph_pc[:, :],
                                        op=mybir.AluOpType.subtract)

                # transpose to [c, patch] (bf16, 4x 128x128)
                pt_ps = pspool.tile([128, 4, 128], mybir.dt.bfloat16,
                                    name="pt_ps", tag="pt_ps")
                for ch in range(2):
                    nc.tensor.transpose(pt_ps[:, ch, :],
                                        ph_pc[:, ch * 128:(ch + 1) * 128],
                                        ident)
                    nc.tensor.transpose(pt_ps[:, 2 + ch, :],
                                        pl_pc[:, ch * 128:(ch + 1) * 128],
                                        ident)
                ph = work.tile([128, 2, 128], mybir.dt.bfloat16, name="ph",
                               tag="ph")
                pl = work.tile([128, 2, 128], mybir.dt.bfloat16, name="pl",
                               tag="pl")
                for ch in range(2):
                    nc.scalar.activation(ph[:, ch, :], pt_ps[:, ch, :],
                                         mybir.ActivationFunctionType.Copy)
                    nc.scalar.activation(pl[:, ch, :], pt_ps[:, 2 + ch, :],
                                         mybir.ActivationFunctionType.Copy)
                return ph, pl

            def stage_b(t, ph, pl):
                sim_ps = simps.tile([128, KCODES], mybir.dt.float32,
                                    name="sim_ps", tag="sim_ps")
                n_mm = [0, 0]
                for ch in range(2):
                    for lhs, ws in ((ph, (whs, wls)), (pl, (whs,))):
                        for w in ws:
                            for nk in range(2):
                                n_mm[nk] += 1
                                nc.tensor.matmul(
                                    sim_ps[:, nk * 512:(nk + 1) * 512],
                                    lhs[:, ch, :],
                                    w[:, ch, nk * 512:(nk + 1) * 512],
                                    start=(n_mm[nk] == 1),
                                    stop=(n_mm[nk] == 6))

                v8 = work.tile([128, 8], mybir.dt.float32, name="v8", tag="v8")
                i8 = work.tile([128, 8], mybir.dt.uint32, name="i8", tag="i8")
                nc.vector.max(v8[:, :], sim_ps[:, :])
                nc.vector.max_index(i8[:, :], v8[:, :], sim_ps[:, :])
                nc.gpsimd.tensor_copy(out=out_sb[:, t:t + 1], in_=i8[:, 0:1])
                if t in (NT // 2 - 1, 3 * NT // 4 - 1):
                    lo = 0 if t == NT // 2 - 1 else NT // 2
                    nc.sync.dma_start(out=out_d[:, lo:t + 1],
                                      in_=out_sb[:, lo:t + 1])

            # software pipeline: transposes/casts of tile t+1 are emitted
            # (and scheduled on PE/ACT) ahead of tile t's matmul block
            prev = None
            for t in range(NT):
                cur = (t, *stage_a(t))
                if prev is not None:
                    stage_b(*prev)
                prev = cur
            stage_b(*prev)

            nc.sync.dma_start(out=out_d[:, 3 * NT // 4:],
                              in_=out_sb[:, 3 * NT // 4:])
    nc.finalize()
    return nc


def _fold_weights(conv_w, proj, codebook):
    """W4 = centered(Wc) @ proj @ l2norm(codebook).T in float64 -> f32."""
    Wc = conv_w.reshape(D, C).T.astype(np.float64)          # [c, d]
    Wc = Wc - Wc.mean(axis=1, keepdims=True)                # center over d
    T2 = Wc @ proj.astype(np.float64)                       # [c, q]
    cb = codebook.astype(np.float64)
    cbn = cb / np.maximum(np.linalg.norm(cb, axis=1, keepdims=True), 1e-12)
    W4 = (T2 @ cbn.T).astype(np.float32)                    # [c, k]
    Wh = W4.astype(ml_dtypes.bfloat16)
    Wl = (W4 - Wh.astype(np.float32)).astype(ml_dtypes.bfloat16)

    def lay(w):  # [c, k] -> [c_local, ch, k]
        return np.ascontiguousarray(
            w.reshape(2, 128, KCODES).transpose(1, 0, 2))
    return lay(Wh), lay(Wl)


def kernel(fbank, conv_w, proj, codebook):
    if "nc" not in _CACHE:
        _CACHE["nc"] = _build_nc()
    nc = _CACHE["nc"]

    conv_w = np.asarray(conv_w, dtype=np.float32)
    proj = np.asarray(proj, dtype=np.float32)
    codebook = np.asarray(codebook, dtype=np.float32)
    wh, wl = _fold_weights(conv_w, proj, codebook)
    fbank = np.ascontiguousarray(np.asarray(fbank), dtype=np.float32)
    in_maps = [
        {"fb": fbank[i * B_LOC:(i + 1) * B_LOC], "wh": wh, "wl": wl}
        for i in range(NCORES)
    ]
    res = run_bass_kernel_spmd(nc, in_maps, core_ids=list(range(NCORES)))

    out = np.empty((B, NPB), dtype=np.int32)
    for i in range(NCORES):
        r = np.asarray(res.results[i]["out"])          # [128, 64] uint32
        # p = 16*w + h_local; t = b_loc*32 + th; n = (16*th + h_local)*8 + w
        o = r.reshape(8, 16, B_LOC, TILES_PER_B)        # [w, h_l, b, th]
        o = o.transpose(2, 3, 1, 0).reshape(B_LOC, NPB)  # [b, th*h_l*w]
        out[i * B_LOC:(i + 1) * B_LOC] = o.astype(np.int32)
    return out


# revision 18
# speedup vs baseline: 1.0062x; 1.0062x over previous
"""Trainium2 Bass kernel for nn_BeatsRandomTokenizer (vq_codebook).

Math: the reference chain (patch-conv 256->512, LayerNorm, proj 512->256,
l2norm, cosine-vs-codebook argmax) folds into a single linear map:
  - LN variance scaling and l2norm are positive per-row scalars -> argmax
    invariant.
  - LN mean subtraction is linear -> folds into the weight.
  sim = P @ W4,  W4 = Wc_centered @ proj @ l2norm(codebook).T  (256 x 1024)
  out = argmax_k sim

Device kernel (per core, 2 of 16 batches):
  64 patch-tiles of 128 patches, software-pipelined. Per tile: one 3-dim
  DMA gathers the patch-major tile [128, 256] (64B runs; partition
  p = 16*w + h so the partition dim merges to a single AP dim), the
  hi/lo bf16 split runs on ACT (cast) + GpSimd (subtract), PE transposes
  the bf16 halves to [c, patch], and 12 bf16 matmuls accumulate the
  f32-accurate product (Ph@Wh + Pl@Wh + Ph@Wl) into PSUM. DVE
  max/max_index read PSUM directly for the argmax over the 1024 codes.
  Measured on trn2: ~212 us HW exec, exact vs the f32 reference.
"""
import numpy as np
import ml_dtypes

import concourse.bass as bass
import concourse.mybir as mybir
import concourse.tile as tile
from concourse import bacc
from concourse.bass_utils import run_bass_kernel_spmd
from concourse.masks import make_identity

B, T, M = 16, 8192, 128
PATCH = 16
D, Q, KCODES = 512, 256, 1024
NCORES = 8
B_LOC = B // NCORES            # 2 batches per core
HB = T // PATCH                # 512 h-patches per batch
WB = M // PATCH                # 8 w-patches
NPB = HB * WB                  # 4096 patches per batch
C = PATCH * PATCH              # 256 patch elements
TILES_PER_B = NPB // 128       # 32 patch-tiles per batch
NT = B_LOC * TILES_PER_B       # 64 tiles per core

_CACHE = {}


def _build_nc():
    nc = bacc.Bacc()
    fb_d = nc.declare_dram_parameter("fb", [B_LOC, T, M], mybir.dt.float32,
                                     isOutput=False)
    wh_d = nc.declare_dram_parameter("wh", [128, 2, KCODES], mybir.dt.bfloat16,
                                     isOutput=False)
    wl_d = nc.declare_dram_parameter("wl", [128, 2, KCODES], mybir.dt.bfloat16,
                                     isOutput=False)
    out_d = nc.declare_dram_parameter("out", [128, NT], mybir.dt.uint32,
                                      isOutput=True)

    # [b, w, t, wp]: per tile in_ = fb_w[b, :, th*256:(th+1)*256, :]
    # -> 3-dim AP [[16,8],[128,256],[1,16]]; partition p = 16*w + h_local
    fb_w = fb_d.rearrange("b t (w wp) -> b w t wp", wp=PATCH)

    with tile.TileContext(nc) as tc:
        with tc.tile_pool(name="singles", bufs=1) as singles, \
             tc.tile_pool(name="io", bufs=6) as io, \
             tc.tile_pool(name="work", bufs=3) as work, \
             tc.tile_pool(name="simpool", bufs=3) as simpool, \
             tc.tile_pool(name="pspool", bufs=2, space="PSUM") as pspool, \
             tc.tile_pool(name="simps", bufs=3, space="PSUM") as simps:
            ident = singles.tile([128, 128], mybir.dt.bfloat16)
            make_identity(nc, ident)
            zero1 = singles.tile([128, 1], mybir.dt.float32)
            nc.vector.memset(zero1, 0.0)

            whs = singles.tile([128, 2, KCODES], mybir.dt.bfloat16)
            wls = singles.tile([128, 2, KCODES], mybir.dt.bfloat16)
            nc.sync.dma_start(out=whs, in_=wh_d[:, :, :])
            nc.sync.dma_start(out=wls, in_=wl_d[:, :, :])

            out_sb = singles.tile([128, NT], mybir.dt.uint32)

            # keep the PE HAM warm during the initial DMA fill (and absorb
            # the Pool->PE identity wait once): dense N=512 matmuls give the
            # activity monitor a sustained-busy window before real work
            scratch = singles.tile([128, 512], mybir.dt.bfloat16)
            nc.vector.memset(scratch, 1.0)
            for wi in range(4):
                warm_ps = pspool.tile([128, 4, 128], mybir.dt.bfloat16,
                                      name=f"warm{wi}", tag="pt_ps")
                nc.tensor.transpose(warm_ps[:, 0, :], ident[:, :], ident[:, :])
            for wi in range(20):
                warm_mm = simps.tile([128, KCODES], mybir.dt.float32,
                                     name=f"warmmm{wi}", tag="sim_ps")
                nc.tensor.matmul(warm_mm[:, 0:512], ident[:, :],
                                 scratch[:, :], start=True, stop=True)

            def stage_a(t):
                b, th = divmod(t, TILES_PER_B)
                ptile = io.tile([128, C], mybir.dt.float32, name="ptile",
                                tag="ptile")
                dma_eng = nc.sync if t % 2 == 0 else nc.scalar
                dma_eng.dma_start(
                    out=ptile,
                    in_=fb_w[b, :, th * 256:(th + 1) * 256, :])

                # hi/lo bf16 split in [patch, c] layout (ACT cast + Pool sub)
                ph_pc = work.tile([128, C], mybir.dt.bfloat16, name="ph_pc",
                                  tag="ph_pc")
                pl_pc = work.tile([128, C], mybir.dt.bfloat16, name="pl_pc",
                                  tag="pl_pc")
                nc.scalar.activation(ph_pc[:, :], ptile[:, :],
                                     mybir.ActivationFunctionType.Copy)
                nc.gpsimd.tensor_tensor(out=pl_pc[:, :], in0=ptile[:, :],
                                        in1=ph_pc[:, :],
                                        op=mybir.AluOpType.subtract)

                # transpose to [c, patch] (bf16, 4x 128x128)
                pt_ps = pspool.tile([128, 4, 128], mybir.dt.bfloat16,
                                    name="pt_ps", tag="pt_ps")
                for ch in range(2):
                    nc.tensor.transpose(pt_ps[:, ch, :],
                                        ph_pc[:, ch * 128:(ch + 1) * 128],
                                        ident)
                    nc.tensor.transpose(pt_ps[:, 2 + ch, :],
                                        pl_pc[:, ch * 128:(ch + 1) * 128],
                                        ident)
                ph = work.tile([128, 2, 128], mybir.dt.bfloat16, name="ph",
                               tag="ph")
                pl = work.tile([128, 2, 128], mybir.dt.bfloat16, name="pl",
                               tag="pl")
                for ch in range(2):
                    nc.scalar.activation(ph[:, ch, :], pt_ps[:, ch, :],
                                         mybir.ActivationFunctionType.Copy)
                    nc.scalar.activation(pl[:, ch, :], pt_ps[:, 2 + ch, :],
                                         mybir.ActivationFunctionType.Copy)
                return ph, pl

            def stage_b(t, ph, pl):
                sim_ps = simps.tile([128, KCODES], mybir.dt.float32,
                                    name="sim_ps", tag="sim_ps")
                n_mm = [0, 0]
                for ch in range(2):
                    for lhs, ws in ((ph, (whs, wls)), (pl, (whs,))):
                        for w in ws:
                            for nk in range(2):
                                n_mm[nk] += 1
                                nc.tensor.matmul(
                                    sim_ps[:, nk * 512:(nk + 1) * 512],
                                    lhs[:, ch, :],
                                    w[:, ch, nk * 512:(nk + 1) * 512],
                                    start=(n_mm[nk] == 1),
                                    stop=(n_mm[nk] == 6))

                v8 = work.tile([128, 8], mybir.dt.float32, name="v8", tag="v8")
                i8 = work.tile([128, 8], mybir.dt.uint32, name="i8", tag="i8")
                nc.vector.max(v8[:, :], sim_ps[:, :])
                nc.vector.max_index(i8[:, :], v8[:, :], sim_ps[:, :])
                nc.gpsimd.tensor_copy(out=out_sb[:, t:t + 1], in_=i8[:, 0:1])
                if t in (NT // 2 - 1, 3 * NT // 4 - 1):
                    lo = 0 if t == NT // 2 - 1 else NT // 2
                    nc.sync.dma_start(out=out_d[:, lo:t + 1],
                                      in_=out_sb[:, lo:t + 1])

            # software pipeline: transposes/casts of tile t+1 are emitted
            # (and scheduled on PE/ACT) ahead of tile t's matmul block
            prev = None
            for t in range(NT):
                cur = (t, *stage_a(t))
                if prev is not None:
                    stage_b(*prev)
                prev = cur
            stage_b(*prev)

            nc.sync.dma_start(out=out_d[:, 3 * NT // 4:],
                              in_=out_sb[:, 3 * NT // 4:])
    nc.finalize()
    return nc


def _fold_weights(conv_w, proj, codebook):
    """W4 = centered(Wc) @ proj @ l2norm(codebook).T in float64 -> f32."""
    Wc = conv_w.reshape(D, C).T.astype(np.float64)          # [c, d]
    Wc = Wc - Wc.mean(axis=1, keepdims=True)                # center over d
    T2 = Wc @ proj.astype(np.float64)                       # [c, q]
    cb = codebook.astype(np.float64)
    cbn = cb / np.maximum(np.linalg.norm(cb, axis=1, keepdims=True), 1e-12)
    W4 = (T2 @ cbn.T).astype(np.float32)                    # [c, k]
    Wh = W4.astype(ml_dtypes.bfloat16)
    Wl = (W4 - Wh.astype(np.float32)).astype(ml_dtypes.bfloat16)

    def lay(w):  # [c, k] -> [c_local, ch, k]
        return np.ascontiguousarray(
            w.reshape(2, 128, KCODES).transpose(1, 0, 2))
    return lay(Wh), lay(Wl)


def kernel(fbank, conv_w, proj, codebook):
    if "nc" not in _CACHE:
        _CACHE["nc"] = _build_nc()
    nc = _CACHE["nc"]

    conv_w = np.asarray(conv_w, dtype=np.float32)
    proj = np.asarray(proj, dtype=np.float32)
    codebook = np.asarray(codebook, dtype=np.float32)
    wh, wl = _fold_weights(conv_w, proj, codebook)
    fbank = np.ascontiguousarray(np.asarray(fbank), dtype=np.float32)
    in_maps = [
        {"fb": fbank[i * B_LOC:(i + 1) * B_LOC], "wh": wh, "wl": wl}
        for i in range(NCORES)
    ]
    res = run_bass_kernel_spmd(nc, in_maps, core_ids=list(range(NCORES)))

    out = np.empty((B, NPB), dtype=np.int32)
    for i in range(NCORES):
        r = np.asarray(res.results[i]["out"])          # [128, 64] uint32
        # p = 16*w + h_local; t = b_loc*32 + th; n = (16*th + h_local)*8 + w
        o = r.reshape(8, 16, B_LOC, TILES_PER_B)        # [w, h_l, b, th]
        o = o.transpose(2, 3, 1, 0).reshape(B_LOC, NPB)  # [b, th*h_l*w]
        out[i * B_LOC:(i + 1) * B_LOC] = o.astype(np.int32)
    return out
